# revision 28
# baseline (speedup 1.0000x reference)
"""GQA attention layer (B=2, T=2048, d_model=2048, 32 Q heads, 8 KV heads,
head_dim=64, RoPE, causal) on 8 Trainium2 NeuronCores.

Sharding: tensor-parallel over KV-head groups. Core c owns Q heads
[4c..4c+4) and KV head c. Projections + RoPE + attention are fully local
per core. The per-core attention outputs y^T (feature-major, bf16) are
exchanged with two AllToAlls (split by head pair so the first one overlaps
the second half of attention); after the exchange the output projection is
token-sharded: each core holds ALL 2048 features for a disjoint slice of
512 tokens and emits that slice of the final output (transposed). The
host does a pure concat + transpose.

Matmuls run as float32r (fp32 storage, 1 PE cycle/row at moving-dim >=
256) except PV (probs/V in bf16) and the output projection (wo + the
exchanged y in bf16) — the bf16 hops sit at the end of the chain so their
~0.4% rounding lands directly on the output, far under the 2e-2 gate.

The PE engine executes its queue in order, so anything that makes a PE
instruction wait on a vector-engine result stalls ALL later matmuls.
Three software pipelines avoid that: (1) each token-tile's RoPE matmuls
are deferred until after the NEXT tile's QKV projection matmuls, so the
DVE PSUM-evacuation copies they depend on are long done when the PE
reaches them; (2) attention runs as one flat beat stream over
(head-pair, window, key-pair) with the score matmuls emitted one beat
ahead globally, so the scalar engine's exp stream (the phase-2
bottleneck, ~178us of ACTIVATE) never drains at window or head-pair
boundaries; (3) each window's normalization is deferred TWO windows (y
is evacuated to SBUF so its PSUM frees immediately), by which time the
DVE reciprocal chain has finished and the rb broadcast matmul issues
without stalling the PE.

Softmax needs no max-subtraction (|scores/sqrt(d)| <~ 6 for these input
scales, exp is safe in fp32). The denominator is accumulated for free by
a ones column appended to V in the PV matmul; the division (via the ~18
bit reciprocal_approx_fast) is applied to y^T before the exchange.

RoPE runs in feature-major layout as q*cosF + shuffle(q)*sinF_signed,
where shuffle (rotate-half) is a permutation matmul on the PE.
"""

import os
import sys

for _p in ("/opt/trn_rl_repo",):
    if _p not in sys.path:
        sys.path.insert(0, _p)

from contextlib import ExitStack

import ml_dtypes
import numpy as np

import concourse.bass as bass  # noqa: F401
import concourse.mybir as mybir
import concourse.tile as tile
from concourse import bacc
from concourse.bass_utils import run_bass_kernel_spmd

F32 = mybir.dt.float32
F32R = mybir.dt.float32r
BF16 = mybir.dt.bfloat16

B = 2
T_FULL = 2048
DM = 2048
HD = 64
N_HEADS = 32
N_KV = 8
N_CORES = 8
QH = N_HEADS // N_KV
QF = QH * HD
SCALE = 1.0 / float(np.sqrt(HD))


def build_gqa(T=T_FULL):
    M = B * T
    KT = DM // 128
    MT = M // 512
    JT = T // 512
    MS = M // N_CORES
    NT = DM // 128

    nc = bacc.Bacc(
        "TRN2", target_bir_lowering=False, debug=False, num_devices=N_CORES
    )

    xT = nc.dram_tensor("xT", [DM, M], F32R, kind="ExternalInput")
    wqT = nc.dram_tensor("wqT", [DM, QF], F32R, kind="ExternalInput")
    wkvT = nc.dram_tensor("wkvT", [DM, 2 * HD], F32R, kind="ExternalInput")
    woP = nc.dram_tensor("woP", [128, NT * 2 * 8 * 128], BF16, kind="ExternalInput")
    cosF = nc.dram_tensor("cosF", [128, T], F32R, kind="ExternalInput")
    sinF = nc.dram_tensor("sinF", [128, T], F32R, kind="ExternalInput")
    pshuf = nc.dram_tensor("pshuf", [128, 128], F32R, kind="ExternalInput")
    pdup = nc.dram_tensor("pdup", [64, 128], F32R, kind="ExternalInput")
    pdups = nc.dram_tensor("pdups", [64, 128], F32R, kind="ExternalInput")
    cmaskM = nc.dram_tensor("cmaskM", [4, 128, 512], BF16, kind="ExternalInput")
    identm = nc.dram_tensor("identm", [64, 64], F32R, kind="ExternalInput")
    onesm = nc.dram_tensor("onesm", [1, 64], F32R, kind="ExternalInput")
    vones = nc.dram_tensor("vones", [128, M // 128, 1], BF16, kind="ExternalInput")
    out = nc.dram_tensor("out", [DM, MS], F32, kind="ExternalOutput")

    with tile.TileContext(nc) as tc, ExitStack() as ctx:
        W = ctx.enter_context(tc.tile_pool(name="weights", bufs=1))
        BIG = ctx.enter_context(tc.tile_pool(name="big", bufs=1))
        EXP = ctx.enter_context(tc.tile_pool(name="exp", bufs=5))
        STR = ctx.enter_context(tc.tile_pool(name="stream", bufs=2))
        PS = ctx.enter_context(tc.tile_pool(name="ps", bufs=4, space="PSUM"))
        DRAM = ctx.enter_context(tc.tile_pool(name="dram", bufs=1, space="DRAM"))
        p1ctx = ExitStack()
        P1 = p1ctx.enter_context(tc.tile_pool(name="p1", bufs=1))

        Exp = mybir.ActivationFunctionType.Exp

        # ---- constant tables (tables on the scalar HWDGE ring, weights +
        # activations on the sync ring so x streaming starts immediately).
        # wq/wkv load in 4 k-chunks so the first matmul only gates on 1/4
        # of the weights plus the first x tile.
        wq_sb = P1.tile([128, KT, QF], F32R, tag="wq")
        wkv_sb = P1.tile([128, KT, 2 * HD], F32R, tag="wkv")
        wq_r = wqT.ap().rearrange("(kt p) f -> p kt f", p=128)
        wkv_r = wkvT.ap().rearrange("(kt p) f -> p kt f", p=128)
        nc.sync.dma_start(out=wq_sb[:, 0:2, :], in_=wq_r[:, 0:2, :])
        nc.sync.dma_start(out=wkv_sb[:, 0:2, :], in_=wkv_r[:, 0:2, :])
        nc.sync.dma_start(out=wq_sb[:, 2:4, :], in_=wq_r[:, 2:4, :])
        nc.sync.dma_start(out=wkv_sb[:, 2:4, :], in_=wkv_r[:, 2:4, :])

        cos_sb = P1.tile([128, T], F32R, tag="cos")
        sin_sb = P1.tile([128, T], F32R, tag="sin")
        pshuf_sb = P1.tile([128, 128], F32R, tag="pshuf")
        pdup_sb = P1.tile([64, 128], F32R, tag="pdup")
        pdups_sb = P1.tile([64, 128], F32R, tag="pdups")
        mask_sb = W.tile([128, 4, 512], BF16, tag="cmaskM")

        def load_tables():
            nc.scalar.dma_start(out=cos_sb, in_=cosF.ap())
            nc.scalar.dma_start(out=sin_sb, in_=sinF.ap())
            nc.scalar.dma_start(out=pshuf_sb, in_=pshuf.ap())
            nc.scalar.dma_start(out=pdup_sb, in_=pdup.ap())
            nc.scalar.dma_start(out=pdups_sb, in_=pdups.ap())
            nc.scalar.dma_start(
                out=mask_sb, in_=cmaskM.ap().rearrange("a p q -> p a q")
            )

        ident = W.tile([64, 64], F32R, tag="ident")
        nc.scalar.dma_start(out=ident, in_=identm.ap())
        ones1 = W.tile([1, 64], F32R, tag="ones1")
        nc.scalar.dma_start(out=ones1, in_=onesm.ap())
        # dummy exp during idle phase 1 so the ~2.7us ACT table load for
        # the exp set doesn't gate the first real softmax activation
        warm = W.tile([1, 64], F32, tag="actwarm")
        nc.scalar.activation(warm, ones1.bitcast(F32), Exp, scale=1.0)

        # ---- persistent activation tensors
        qrope = [
            BIG.tile([128, M], F32R, tag=f"qrope{f}", name=f"qrope{f}")
            for f in range(2)
        ]
        ktdup = BIG.tile([128, M], F32R, tag="ktdup")
        vaug_all = BIG.tile([128, M // 128, HD + 1], BF16, tag="vaug")
        vaug = [vaug_all[:, i, :] for i in range(M // 128)]
        nc.scalar.dma_start(out=vaug_all[:, :, HD:HD + 1], in_=vones.ap())

        a2a_in = [
            DRAM.tile([N_CORES, 128, MS], BF16, tag=f"a2a_in{f}", name=f"a2a_in{f}")
            for f in range(2)
        ]
        a2a_out = [
            DRAM.tile([N_CORES, 128, MS], BF16, tag=f"a2a_out{f}", name=f"a2a_out{f}")
            for f in range(2)
        ]

        # ---- phase 1: QKV projections + RoPE + V transpose.
        # The RoPE/transpose matmul block for tile mi is issued after the
        # projection matmuls of tile mi+1 (PE executes in order, and the
        # rope matmuls wait on DVE PSUM-evacuation copies — deferring them
        # keeps the PE dense).
        rope_pend = []

        def rope_block(st):
            qp2, kv_sb, v_sb, ms, tsl = st
            qps = [qp2[:, 512 * f:512 * (f + 1)] for f in range(2)]
            for f in range(2):
                q_sb = P1.tile([128, 512], F32R, tag="q_sb", bufs=2)
                nc.vector.tensor_copy(q_sb, qps[f])
                qs_ps = PS.tile([128, 512], F32, tag="ps1")
                nc.tensor.matmul(
                    qs_ps, pshuf_sb.bitcast(F32R), q_sb.bitcast(F32R),
                    start=True, stop=True,
                )
                t1 = P1.tile([128, 512], F32R, tag="t1", bufs=2)
                nc.vector.tensor_mul(t1, q_sb, cos_sb[:, tsl:tsl + 512])
                t2 = P1.tile([128, 512], F32R, tag="t2", bufs=2)
                nc.vector.tensor_mul(t2, qs_ps, sin_sb[:, tsl:tsl + 512])
                nc.vector.tensor_add(qrope[f][:, ms:ms + 512], t1, t2)
            kd_ps = PS.tile([128, 512], F32, tag="ps1")
            nc.tensor.matmul(
                kd_ps, pdup_sb.bitcast(F32R), kv_sb[0:64, :].bitcast(F32R),
                start=True, stop=True,
            )
            ks_ps = PS.tile([128, 512], F32, tag="ps1")
            nc.tensor.matmul(
                ks_ps, pdups_sb.bitcast(F32R), kv_sb[0:64, :].bitcast(F32R),
                start=True, stop=True,
            )
            t1 = P1.tile([128, 512], F32R, tag="t1", bufs=2)
            nc.vector.tensor_mul(t1, kd_ps, cos_sb[:, tsl:tsl + 512])
            t2 = P1.tile([128, 512], F32R, tag="t2", bufs=2)
            nc.vector.tensor_mul(t2, ks_ps, sin_sb[:, tsl:tsl + 512])
            nc.vector.tensor_add(ktdup[:, ms:ms + 512], t1, t2)
            for t in range(4):
                gi = ms // 128 + t
                vt_ps = PS.tile([128, 64], F32R, tag="ps1")
                nc.tensor.transpose(
                    vt_ps, v_sb[:, 128 * t:128 * (t + 1)], ident
                )
                with nc.allow_low_precision(reason="bf16 V"):
                    nc.vector.tensor_copy(vaug[gi][:, 0:HD], vt_ps)

        for mi in range(MT):
            ms = 512 * mi
            tsl = ms % T
            xts = []
            for kg in range(KT // 4):
                xt = P1.tile([128, 4, 512], F32R, tag="xt", bufs=5)
                if mi == 0 and kg == 0:
                    # finest granularity for the very first tile so the
                    # first projection matmul gates on 256KB, not 1MB
                    for a in range(4):
                        nc.sync.dma_start(
                            out=xt[:, a, :],
                            in_=xT.ap()[128 * a:128 * (a + 1), ms:ms + 512],
                        )
                else:
                    nc.sync.dma_start(
                        out=xt,
                        in_=xT.ap()[512 * kg:512 * (kg + 1), ms:ms + 512]
                        .rearrange("(a p) m -> p a m", p=128),
                    )
                xts.append(xt)
                if mi == 0 and kg < 3:
                    # stream the remaining weight chunks between the x tiles
                    nc.sync.dma_start(
                        out=wq_sb[:, 4 * (kg + 1):4 * (kg + 2), :],
                        in_=wq_r[:, 4 * (kg + 1):4 * (kg + 2), :],
                    )
                    nc.sync.dma_start(
                        out=wkv_sb[:, 4 * (kg + 1):4 * (kg + 2), :],
                        in_=wkv_r[:, 4 * (kg + 1):4 * (kg + 2), :],
                    )
            if mi == 0:
                load_tables()
            qp2 = PS.tile([128, 1024], F32, tag="ps2", bufs=2, name="qp2")
            qps = [qp2[:, 512 * f:512 * (f + 1)] for f in range(2)]
            kvps = PS.tile([128, 512], F32, tag="ps1")
            for k in range(KT):
                xk = xts[k // 4][:, k % 4, :].bitcast(F32R)
                st, sp = k == 0, k == KT - 1
                for f in range(2):
                    nc.tensor.matmul(
                        qps[f],
                        wq_sb[:, k, 128 * f:128 * (f + 1)].bitcast(F32R),
                        xk, start=st, stop=sp,
                    )
                nc.tensor.matmul(
                    kvps, wkv_sb[:, k, :].bitcast(F32R), xk, start=st, stop=sp
                )
            # evacuate kv immediately (DVE, no PE dependency)
            kv_sb = P1.tile([128, 512], F32R, tag="kv_sb", bufs=2)
            nc.vector.tensor_copy(kv_sb, kvps)
            v_sb = P1.tile([64, 512], F32R, tag="v_sb", bufs=2)
            nc.vector.tensor_copy(v_sb, kvps[64:128, :])
            rope_pend.append((qp2, kv_sb, v_sb, ms, tsl))
            if mi > 0:
                rope_block(rope_pend.pop(0))
        rope_block(rope_pend.pop(0))

        p1ctx.close()

        # ---- wo prefetch: host-packed contiguous bf16 strip, one big DMA on
        # the sync ring. Queued after all phase-1 x loads, lands early in
        # phase 2 (needed at ~the first AllToAll). The pool opens after P1
        # closes so its 8MB reuses phase-1 SBUF space.
        p3ctx = ExitStack()
        P3 = p3ctx.enter_context(tc.tile_pool(name="p3", bufs=1))
        wo_sb = P3.tile([128, NT, 2, 8, 128], BF16, tag="wo_sb", name="wo_sb")
        nc.sync.dma_start(
            out=wo_sb,
            in_=woP.ap().rearrange(
                "p (n two s c) -> p n two s c", n=NT, two=2, s=8
            ),
        )

        # ---- phase 2: attention, head-pair (fp) outer so the first
        # AllToAll overlaps the second head-pair's compute. Each window's
        # normalization is deferred into the next window (norm_pend) so its
        # rb matmul never stalls the PE waiting on the DVE reciprocal.
        norm_pend = []

        def norm_block(st):
            y_sbs, mq0, fp, r_sbs = st
            yt = STR.tile([128, 512], BF16, tag="yt", bufs=3)
            for hh in range(2):
                rb_ps = PS.tile([64, 512], F32, tag="ps1", bufs=4)
                nc.tensor.matmul(
                    rb_ps, ones1.bitcast(F32R),
                    r_sbs[hh].bitcast(F32R),
                    start=True, stop=True,
                )
                rb_sb = STR.tile([64, 512], F32R, tag="rb_sb", bufs=2)
                nc.vector.tensor_copy(rb_sb, rb_ps)
                with nc.allow_low_precision(reason="bf16 y"):
                    nc.vector.tensor_mul(
                        yt[64 * hh:64 * (hh + 1), :],
                        y_sbs[hh], rb_sb,
                    )
            lo = mq0
            while lo < mq0 + 512:
                s = lo // MS
                hi = min(mq0 + 512, (s + 1) * MS)
                nc.sync.dma_start(
                    out=a2a_in[fp][s, :, lo - s * MS:hi - s * MS],
                    in_=yt[:, lo - mq0:hi - mq0],
                )
                lo = hi

        def emit_a2a(fp):
            if os.environ.get("GQA_NO_CC"):
                nc.sync.dma_start(out=a2a_out[fp].opt(), in_=a2a_in[fp].opt())
            else:
                nc.gpsimd.collective_compute(
                    "AllToAll",
                    mybir.AluOpType.bypass,
                    replica_groups=[list(range(N_CORES))],
                    ins=[a2a_in[fp].opt()],
                    outs=[a2a_out[fp].opt()],
                )

        def emit_scores(fp, j, b, ip):
            mq0 = T * b + 512 * j
            s2 = [
                PS.tile([128, 1024], F32, tag="ps2", bufs=2, name="s2")
                for _ in range(2)
            ]
            for hh in range(2):
                for di in range(2):
                    mk0 = T * b + 128 * (2 * ip + di)
                    nc.tensor.matmul(
                        s2[hh][:, 512 * di:512 * (di + 1)],
                        ktdup[64 * hh:64 * (hh + 1), mk0:mk0 + 128]
                        .bitcast(F32R),
                        qrope[fp][64 * hh:64 * (hh + 1),
                                  mq0:mq0 + 512].bitcast(F32R),
                        start=True, stop=True,
                        tile_position=(64 * hh, 0),
                    )
            return s2

        # flat beat stream over (fp, window, key-pair) with the scores
        # pipelined one beat ahead GLOBALLY (across window and head-pair
        # boundaries), so the scalar engine's exp stream never drains
        beats = []
        # windows ordered large-j interleaved with small-j so the per-window
        # normalization overhead (a PE matmul pair per window start) stays
        # spread out instead of bunching up in the small-window tail
        win_order = [(3, 0), (2, 0), (3, 1), (1, 0), (2, 1), (0, 0),
                     (1, 1), (0, 1)]
        assert JT == 4
        for fp in range(2):
            for j, b in win_order:
                ntk = 4 * (j + 1)
                for ip in range(ntk // 2):
                    beats.append((fp, j, b, ip, ntk))
        fp0_norms = 0
        pvps = None
        s2 = emit_scores(*beats[0][:4])
        for idx, (fp, j, b, ip, ntk) in enumerate(beats):
            mq0 = T * b + 512 * j
            s2_next = (
                emit_scores(*beats[idx + 1][:4])
                if idx + 1 < len(beats) else None
            )
            if ip == 0:
                pvps = [
                    PS.tile([HD + 1, 512], F32, tag="ps1", name="pvps")
                    for _ in range(2)
                ]
                if len(norm_pend) > 1:
                    # normalization deferred TWO windows: by the time the PE
                    # reaches the rb broadcast matmul, the DVE reciprocal
                    # chain finished long ago, so the PE stream never stalls
                    nb = norm_pend.pop(0)
                    norm_block(nb)
                    if nb[2] == 0:
                        fp0_norms += 1
                        if fp0_norms == 2 * JT:
                            emit_a2a(0)
            elif idx == len(beats) - 1 and norm_pend:
                # drain one pending normalization inside the final beat so
                # the end-of-phase flush doesn't delay the second AllToAll
                nb = norm_pend.pop(0)
                norm_block(nb)
                if nb[2] == 0:
                    fp0_norms += 1
                    if fp0_norms == 2 * JT:
                        emit_a2a(0)
            for hh in range(2):
                e_sb = EXP.tile([128, 1024], BF16, tag="e_sb")
                with nc.allow_low_precision(reason="bf16 probs"):
                    nc.scalar.activation(e_sb, s2[hh], Exp, scale=SCALE)
                for di in range(2):
                    i = 2 * ip + di
                    rel = i - 4 * j
                    if rel >= 0:
                        with nc.allow_low_precision(reason="bf16 probs"):
                            nc.vector.tensor_mul(
                                e_sb[:, 512 * di:512 * (di + 1)],
                                e_sb[:, 512 * di:512 * (di + 1)],
                                mask_sb[:, rel, :],
                            )
                for di in range(2):
                    i = 2 * ip + di
                    gi = (T * b) // 128 + i
                    nc.tensor.matmul(
                        pvps[hh], vaug[gi].bitcast(BF16),
                        e_sb[:, 512 * di:512 * (di + 1)].bitcast(BF16),
                        start=(i == 0), stop=(i == ntk - 1),
                    )
            if ip == ntk // 2 - 1:
                # window done (all DVE, off the PE path): reciprocal of the
                # partition-0 denominator row, and evacuation of y to SBUF
                # so the pvps PSUM slots free without waiting for the
                # (deferred) normalization.
                r_sbs = []
                y_sbs = []
                for hh in range(2):
                    d_sb = STR.tile([1, 512], F32, tag="d_sb", bufs=2)
                    nc.vector.tensor_copy(d_sb, pvps[hh][HD:HD + 1, :])
                    r32 = STR.tile([1, 512], F32, tag="r32", bufs=2)
                    nc.vector.reciprocal_approx_fast(r32, d_sb)
                    r_sb = STR.tile([1, 512], F32R, tag="r_sb", bufs=6)
                    nc.vector.tensor_copy(r_sb, r32)
                    r_sbs.append(r_sb)
                    y_sb = STR.tile([64, 512], BF16, tag="y_sb", bufs=6)
                    with nc.allow_low_precision(reason="bf16 y"):
                        nc.vector.tensor_copy(y_sb, pvps[hh][0:HD, :])
                    y_sbs.append(y_sb)
                norm_pend.append((y_sbs, mq0, fp, r_sbs))
            s2 = s2_next
        while norm_pend:
            nb = norm_pend.pop(0)
            norm_block(nb)
            if nb[2] == 0:
                fp0_norms += 1
                if fp0_norms == 2 * JT:
                    emit_a2a(0)
        emit_a2a(1)

        # ---- phase 3: token-sharded output projection (emits out^T).
        # ytf loads on the sync ring; wo is already resident (prefetched).
        ytf = {}
        for s in range(N_CORES):
            yt_sb = P3.tile([128, MS], BF16, tag=f"ytf{2 * s}",
                            name=f"ytf{2 * s}")
            nc.sync.dma_start(out=yt_sb, in_=a2a_out[0][s, :, :])
            ytf[2 * s] = yt_sb
        # even k-tiles (from the first AllToAll): accumulate into SBUF
        # partials while the second AllToAll is in flight
        oe_sbs = []
        for n in range(NT):
            oe_ps = PS.tile([128, MS], F32, tag="ps1", name="oe_ps")
            for s in range(N_CORES):
                nc.tensor.matmul(
                    oe_ps, wo_sb[:, n, 0, s, :].bitcast(BF16),
                    ytf[2 * s].bitcast(BF16),
                    start=(s == 0), stop=(s == N_CORES - 1),
                )
            oe_sb = P3.tile([128, MS], BF16, tag=f"oe{n}", bufs=1,
                            name=f"oe{n}")
            with nc.allow_low_precision(reason="bf16 partial"):
                nc.vector.tensor_copy(oe_sb, oe_ps)
            oe_sbs.append(oe_sb)
        for s in range(N_CORES):
            yt_sb = P3.tile([128, MS], BF16, tag=f"ytf{2 * s + 1}",
                            name=f"ytf{2 * s + 1}")
            nc.sync.dma_start(out=yt_sb, in_=a2a_out[1][s, :, :])
            ytf[2 * s + 1] = yt_sb
        for n in range(NT):
            oo_ps = PS.tile([128, MS], F32, tag="ps1", name="oo_ps")
            for s in range(N_CORES):
                nc.tensor.matmul(
                    oo_ps, wo_sb[:, n, 1, s, :].bitcast(BF16),
                    ytf[2 * s + 1].bitcast(BF16),
                    start=(s == 0), stop=(s == N_CORES - 1),
                )
            ot_sb = P3.tile([128, MS], F32, tag="ot_sb", bufs=3)
            nc.vector.tensor_add(ot_sb, oo_ps, oe_sbs[n])
            nc.sync.dma_start(
                out=out.ap()[128 * n:128 * (n + 1), :], in_=ot_sb
            )
        p3ctx.close()

    nc.finalize()
    return nc


def make_inputs(x, cos, sin, wq, wk, wv, wo):
    """Host-side sharding/layout prep. Returns in_maps for the 8 cores."""
    Bx, T, _ = x.shape
    M = Bx * T
    NT = DM // 128
    xT = np.ascontiguousarray(x.reshape(M, DM).T)
    # wo packed for a single contiguous DMA into the phase-3 SBUF layout:
    # woP[p, n, two, s, c] = wo.T[s*256 + two*128 + p, n*128 + c], bf16
    woT = wo.T.reshape(8, 2, 128, NT, 128)
    woP = np.ascontiguousarray(
        woT.transpose(2, 3, 1, 0, 4).reshape(128, -1)
    ).astype(ml_dtypes.bfloat16)
    sgn = np.concatenate([-np.ones(32, np.float32), np.ones(32, np.float32)])
    cosF = np.ascontiguousarray(np.tile(cos.T, (2, 1))).astype(np.float32)
    sinF = np.ascontiguousarray(np.tile(sin.T * sgn[:, None], (2, 1))).astype(
        np.float32
    )
    pshuf = np.zeros((128, 128), np.float32)
    for m in range(128):
        pshuf[64 * (m // 64) + (m % 64 + 32) % 64, m] = 1.0
    pdup = np.zeros((64, 128), np.float32)
    pdups = np.zeros((64, 128), np.float32)
    for m in range(128):
        pdup[m % 64, m] = 1.0
        pdups[(m % 64 + 32) % 64, m] = 1.0
    p = np.arange(128)[:, None]
    q = np.arange(512)[None, :]
    cmaskM = np.stack(
        [(128 * r + p <= q).astype(np.float32) for r in range(4)]
    ).astype(ml_dtypes.bfloat16)
    in_maps = []
    for c in range(N_CORES):
        wqT = np.ascontiguousarray(wq[QF * c:QF * (c + 1), :].T)
        wkvT = np.ascontiguousarray(
            np.concatenate(
                [wk[HD * c:HD * (c + 1), :], wv[HD * c:HD * (c + 1), :]],
                axis=0,
            ).T
        )
        in_maps.append(
            {
                "xT": xT, "wqT": wqT, "wkvT": wkvT, "woP": woP,
                "cosF": cosF, "sinF": sinF, "pshuf": pshuf,
                "pdup": pdup, "pdups": pdups, "cmaskM": cmaskM,
                "identm": np.eye(64, dtype=np.float32),
                "onesm": np.ones((1, 64), np.float32),
                "vones": np.ones((128, M // 128, 1), ml_dtypes.bfloat16),
            }
        )
    return in_maps


_NC_CACHE = {}


def get_nc(T=T_FULL):
    if T not in _NC_CACHE:
        _NC_CACHE[T] = build_gqa(T)
    return _NC_CACHE[T]


def kernel(x, cos, sin, wq, wk, wv, wo, _trace=False):
    x = np.asarray(x, np.float32)
    nc = get_nc(x.shape[1])
    in_maps = make_inputs(
        x,
        np.asarray(cos, np.float32),
        np.asarray(sin, np.float32),
        np.asarray(wq, np.float32),
        np.asarray(wk, np.float32),
        np.asarray(wv, np.float32),
        np.asarray(wo, np.float32),
    )
    res = run_bass_kernel_spmd(nc, in_maps, list(range(N_CORES)), trace=_trace)
    # each core returns out^T (2048, M/8) for its disjoint token slice
    outs = [np.asarray(res.results[c]["out"]).T for c in range(N_CORES)]
    full = np.concatenate(outs, axis=0)
    Bx, T, _ = x.shape
    out = np.ascontiguousarray(full).reshape(Bx, T, DM).astype(np.float32)
    if _trace:
        return out, res
    return out


# revision 30
# speedup vs baseline: 1.1214x; 1.1214x over previous
"""GQA attention layer (B=2, T=2048, d_model=2048, 32 Q heads, 8 KV heads,
head_dim=64, RoPE, causal) on 8 Trainium2 NeuronCores.

Sharding: tensor-parallel over KV-head groups. Core c owns Q heads
[4c..4c+4) and KV head c. Projections + RoPE + attention are fully local
per core. The per-core attention outputs y^T (feature-major, bf16) are
exchanged with two AllToAlls (split by head pair so the first one overlaps
the second half of attention); after the exchange the output projection is
token-sharded: each core holds ALL 2048 features for a disjoint slice of
512 tokens and emits that slice of the final output (transposed). The
host does a pure concat + transpose.

Matmuls run as float32r (fp32 storage, 1 PE cycle/row at moving-dim >=
256) except PV (probs/V in bf16) and the output projection (wo + the
exchanged y in bf16) — the bf16 hops sit at the end of the chain so their
~0.4% rounding lands directly on the output, far under the 2e-2 gate.

The PE engine executes its queue in order, so anything that makes a PE
instruction wait on a vector-engine result stalls ALL later matmuls.
Three software pipelines avoid that: (1) each token-tile's RoPE matmuls
are deferred until after the NEXT tile's QKV projection matmuls, so the
DVE PSUM-evacuation copies they depend on are long done when the PE
reaches them; (2) attention runs as one flat beat stream over
(head-pair, window, key-pair) with the score matmuls emitted one beat
ahead globally, so the scalar engine's exp stream (the phase-2
bottleneck, ~178us of ACTIVATE) never drains at window or head-pair
boundaries; (3) each window's normalization is deferred TWO windows (y
is evacuated to SBUF so its PSUM frees immediately), by which time the
DVE reciprocal chain has finished and the rb broadcast matmul issues
without stalling the PE.

Softmax needs no max-subtraction (|scores/sqrt(d)| <~ 6 for these input
scales, exp is safe in fp32). The denominator is accumulated for free by
a ones column appended to V in the PV matmul; the division (via the ~18
bit reciprocal_approx_fast) is applied to y^T before the exchange.

RoPE runs in feature-major layout as q*cosF + shuffle(q)*sinF_signed,
where shuffle (rotate-half) is a permutation matmul on the PE.
"""

import os
import sys

for _p in ("/opt/trn_rl_repo",):
    if _p not in sys.path:
        sys.path.insert(0, _p)

from contextlib import ExitStack

import ml_dtypes
import numpy as np

import concourse.bass as bass  # noqa: F401
import concourse.mybir as mybir
import concourse.tile as tile
from concourse import bacc
from concourse.bass_utils import run_bass_kernel_spmd

F32 = mybir.dt.float32
F32R = mybir.dt.float32r
BF16 = mybir.dt.bfloat16

B = 2
T_FULL = 2048
DM = 2048
HD = 64
N_HEADS = 32
N_KV = 8
N_CORES = 8
QH = N_HEADS // N_KV
QF = QH * HD
SCALE = 1.0 / float(np.sqrt(HD))


def build_gqa(T=T_FULL):
    M = B * T
    KT = DM // 128
    MT = M // 512
    JT = T // 512
    MS = M // N_CORES
    NT = DM // 128

    nc = bacc.Bacc(
        "TRN2", target_bir_lowering=False, debug=False, num_devices=N_CORES
    )

    xT = nc.dram_tensor("xT", [DM, M], F32R, kind="ExternalInput")
    wqT = nc.dram_tensor("wqT", [DM, QF], F32R, kind="ExternalInput")
    wkvT = nc.dram_tensor("wkvT", [DM, 2 * HD], F32R, kind="ExternalInput")
    woP = nc.dram_tensor("woP", [128, NT * 2 * 8 * 128], BF16, kind="ExternalInput")
    cosF = nc.dram_tensor("cosF", [128, T], F32R, kind="ExternalInput")
    sinF = nc.dram_tensor("sinF", [128, T], F32R, kind="ExternalInput")
    pshuf = nc.dram_tensor("pshuf", [128, 128], F32R, kind="ExternalInput")
    pdup = nc.dram_tensor("pdup", [64, 128], F32R, kind="ExternalInput")
    pdups = nc.dram_tensor("pdups", [64, 128], F32R, kind="ExternalInput")
    cmaskM = nc.dram_tensor("cmaskM", [4, 128, 512], BF16, kind="ExternalInput")
    identm = nc.dram_tensor("identm", [64, 64], F32R, kind="ExternalInput")
    onesm = nc.dram_tensor("onesm", [1, 64], F32R, kind="ExternalInput")
    vones = nc.dram_tensor("vones", [128, M // 128, 1], BF16, kind="ExternalInput")
    out = nc.dram_tensor("out", [DM, MS], F32, kind="ExternalOutput")

    with tile.TileContext(nc) as tc, ExitStack() as ctx:
        W = ctx.enter_context(tc.tile_pool(name="weights", bufs=1))
        BIG = ctx.enter_context(tc.tile_pool(name="big", bufs=1))
        EXP = ctx.enter_context(tc.tile_pool(name="exp", bufs=5))
        STR = ctx.enter_context(tc.tile_pool(name="stream", bufs=2))
        PS = ctx.enter_context(tc.tile_pool(name="ps", bufs=4, space="PSUM"))
        DRAM = ctx.enter_context(tc.tile_pool(name="dram", bufs=1, space="DRAM"))
        p1ctx = ExitStack()
        P1 = p1ctx.enter_context(tc.tile_pool(name="p1", bufs=1))

        Exp = mybir.ActivationFunctionType.Exp

        # ---- constant tables (tables on the scalar HWDGE ring, weights +
        # activations on the sync ring so x streaming starts immediately).
        # wq/wkv load in 4 k-chunks so the first matmul only gates on 1/4
        # of the weights plus the first x tile.
        wq_sb = P1.tile([128, KT, QF], F32R, tag="wq")
        wkv_sb = P1.tile([128, KT, 2 * HD], F32R, tag="wkv")
        wq_r = wqT.ap().rearrange("(kt p) f -> p kt f", p=128)
        wkv_r = wkvT.ap().rearrange("(kt p) f -> p kt f", p=128)
        nc.sync.dma_start(out=wq_sb[:, 0:2, :], in_=wq_r[:, 0:2, :])
        nc.sync.dma_start(out=wkv_sb[:, 0:2, :], in_=wkv_r[:, 0:2, :])
        nc.sync.dma_start(out=wq_sb[:, 2:4, :], in_=wq_r[:, 2:4, :])
        nc.sync.dma_start(out=wkv_sb[:, 2:4, :], in_=wkv_r[:, 2:4, :])

        cos_sb = P1.tile([128, T], F32R, tag="cos")
        sin_sb = P1.tile([128, T], F32R, tag="sin")
        pshuf_sb = P1.tile([128, 128], F32R, tag="pshuf")
        pdup_sb = P1.tile([64, 128], F32R, tag="pdup")
        pdups_sb = P1.tile([64, 128], F32R, tag="pdups")
        mask_sb = W.tile([128, 4, 512], BF16, tag="cmaskM")

        def load_tables():
            nc.scalar.dma_start(out=cos_sb, in_=cosF.ap())
            nc.scalar.dma_start(out=sin_sb, in_=sinF.ap())
            nc.scalar.dma_start(out=pshuf_sb, in_=pshuf.ap())
            nc.scalar.dma_start(out=pdup_sb, in_=pdup.ap())
            nc.scalar.dma_start(out=pdups_sb, in_=pdups.ap())
            nc.scalar.dma_start(
                out=mask_sb, in_=cmaskM.ap().rearrange("a p q -> p a q")
            )

        ident = W.tile([64, 64], F32R, tag="ident")
        nc.scalar.dma_start(out=ident, in_=identm.ap())
        ones1 = W.tile([1, 64], F32R, tag="ones1")
        nc.scalar.dma_start(out=ones1, in_=onesm.ap())
        # dummy exp during idle phase 1 so the ~2.7us ACT table load for
        # the exp set doesn't gate the first real softmax activation
        warm = W.tile([1, 64], F32, tag="actwarm")
        nc.scalar.activation(warm, ones1.bitcast(F32), Exp, scale=1.0)

        # ---- persistent activation tensors
        qrope = [
            BIG.tile([128, M], F32R, tag=f"qrope{f}", name=f"qrope{f}")
            for f in range(2)
        ]
        ktdup = BIG.tile([128, M], F32R, tag="ktdup")
        vaug_all = BIG.tile([128, M // 128, HD + 1], BF16, tag="vaug")
        vaug = [vaug_all[:, i, :] for i in range(M // 128)]
        nc.scalar.dma_start(out=vaug_all[:, :, HD:HD + 1], in_=vones.ap())

        a2a_in = [
            DRAM.tile([N_CORES, 128, MS], BF16, tag=f"a2a_in{f}", name=f"a2a_in{f}")
            for f in range(2)
        ]
        a2a_out = [
            DRAM.tile([N_CORES, 128, MS], BF16, tag=f"a2a_out{f}", name=f"a2a_out{f}")
            for f in range(2)
        ]

        # ---- phase 1: QKV projections + RoPE + V transpose.
        # The RoPE/transpose matmul block for tile mi is issued after the
        # projection matmuls of tile mi+1 (PE executes in order, and the
        # rope matmuls wait on DVE PSUM-evacuation copies — deferring them
        # keeps the PE dense).
        rope_pend = []

        def rope_block(st):
            qp2, kv_sb, v_sb, ms, tsl = st
            qps = [qp2[:, 512 * f:512 * (f + 1)] for f in range(2)]
            for f in range(2):
                q_sb = P1.tile([128, 512], F32R, tag="q_sb", bufs=2)
                nc.vector.tensor_copy(q_sb, qps[f])
                qs_ps = PS.tile([128, 512], F32, tag="ps1")
                nc.tensor.matmul(
                    qs_ps, pshuf_sb.bitcast(F32R), q_sb.bitcast(F32R),
                    start=True, stop=True,
                )
                t1 = P1.tile([128, 512], F32R, tag="t1", bufs=2)
                nc.vector.tensor_mul(t1, q_sb, cos_sb[:, tsl:tsl + 512])
                t2 = P1.tile([128, 512], F32R, tag="t2", bufs=2)
                nc.vector.tensor_mul(t2, qs_ps, sin_sb[:, tsl:tsl + 512])
                nc.vector.tensor_add(qrope[f][:, ms:ms + 512], t1, t2)
            kd_ps = PS.tile([128, 512], F32, tag="ps1")
            nc.tensor.matmul(
                kd_ps, pdup_sb.bitcast(F32R), kv_sb[0:64, :].bitcast(F32R),
                start=True, stop=True,
            )
            ks_ps = PS.tile([128, 512], F32, tag="ps1")
            nc.tensor.matmul(
                ks_ps, pdups_sb.bitcast(F32R), kv_sb[0:64, :].bitcast(F32R),
                start=True, stop=True,
            )
            t1 = P1.tile([128, 512], F32R, tag="t1", bufs=2)
            nc.vector.tensor_mul(t1, kd_ps, cos_sb[:, tsl:tsl + 512])
            t2 = P1.tile([128, 512], F32R, tag="t2", bufs=2)
            nc.vector.tensor_mul(t2, ks_ps, sin_sb[:, tsl:tsl + 512])
            nc.vector.tensor_add(ktdup[:, ms:ms + 512], t1, t2)
            for t in range(4):
                gi = ms // 128 + t
                vt_ps = PS.tile([128, 64], F32R, tag="ps1")
                nc.tensor.transpose(
                    vt_ps, v_sb[:, 128 * t:128 * (t + 1)], ident
                )
                with nc.allow_low_precision(reason="bf16 V"):
                    nc.vector.tensor_copy(vaug[gi][:, 0:HD], vt_ps)

        for mi in range(MT):
            ms = 512 * mi
            tsl = ms % T
            xts = []
            for kg in range(KT // 4):
                xt = P1.tile([128, 4, 512], F32R, tag="xt", bufs=5)
                if mi == 0 and kg == 0:
                    # finest granularity for the very first tile so the
                    # first projection matmul gates on 256KB, not 1MB
                    for a in range(4):
                        nc.sync.dma_start(
                            out=xt[:, a, :],
                            in_=xT.ap()[128 * a:128 * (a + 1), ms:ms + 512],
                        )
                else:
                    nc.sync.dma_start(
                        out=xt,
                        in_=xT.ap()[512 * kg:512 * (kg + 1), ms:ms + 512]
                        .rearrange("(a p) m -> p a m", p=128),
                    )
                xts.append(xt)
                if mi == 0 and kg < 3:
                    # stream the remaining weight chunks between the x tiles
                    nc.sync.dma_start(
                        out=wq_sb[:, 4 * (kg + 1):4 * (kg + 2), :],
                        in_=wq_r[:, 4 * (kg + 1):4 * (kg + 2), :],
                    )
                    nc.sync.dma_start(
                        out=wkv_sb[:, 4 * (kg + 1):4 * (kg + 2), :],
                        in_=wkv_r[:, 4 * (kg + 1):4 * (kg + 2), :],
                    )
            if mi == 0:
                load_tables()
            qp2 = PS.tile([128, 1024], F32, tag="ps2", bufs=2, name="qp2")
            qps = [qp2[:, 512 * f:512 * (f + 1)] for f in range(2)]
            kvps = PS.tile([128, 512], F32, tag="ps1")
            for k in range(KT):
                xk = xts[k // 4][:, k % 4, :].bitcast(F32R)
                st, sp = k == 0, k == KT - 1
                for f in range(2):
                    nc.tensor.matmul(
                        qps[f],
                        wq_sb[:, k, 128 * f:128 * (f + 1)].bitcast(F32R),
                        xk, start=st, stop=sp,
                    )
                nc.tensor.matmul(
                    kvps, wkv_sb[:, k, :].bitcast(F32R), xk, start=st, stop=sp
                )
            # evacuate kv immediately (DVE, no PE dependency)
            kv_sb = P1.tile([128, 512], F32R, tag="kv_sb", bufs=2)
            nc.vector.tensor_copy(kv_sb, kvps)
            v_sb = P1.tile([64, 512], F32R, tag="v_sb", bufs=2)
            nc.vector.tensor_copy(v_sb, kvps[64:128, :])
            rope_pend.append((qp2, kv_sb, v_sb, ms, tsl))
            if mi > 0:
                rope_block(rope_pend.pop(0))
        rope_block(rope_pend.pop(0))

        p1ctx.close()

        # ---- wo prefetch: host-packed contiguous bf16 strip, one big DMA on
        # the sync ring. Queued after all phase-1 x loads, lands early in
        # phase 2 (needed at ~the first AllToAll). The pool opens after P1
        # closes so its 8MB reuses phase-1 SBUF space.
        p3ctx = ExitStack()
        P3 = p3ctx.enter_context(tc.tile_pool(name="p3", bufs=1))
        wo_sb = P3.tile([128, NT, 2, 8, 128], BF16, tag="wo_sb", name="wo_sb")
        nc.sync.dma_start(
            out=wo_sb,
            in_=woP.ap().rearrange(
                "p (n two s c) -> p n two s c", n=NT, two=2, s=8
            ),
        )

        # ---- phase 2: attention, head-pair (fp) outer so the first
        # AllToAll overlaps the second head-pair's compute. Each window's
        # normalization is deferred into the next window (norm_pend) so its
        # rb matmul never stalls the PE waiting on the DVE reciprocal.
        norm_pend = []

        def norm_block(st):
            y_sbs, mq0, fp, r_sbs = st
            yt = STR.tile([128, 512], BF16, tag="yt", bufs=3)
            for hh in range(2):
                rb_ps = PS.tile([64, 512], F32, tag="ps1", bufs=4)
                nc.tensor.matmul(
                    rb_ps, ones1.bitcast(F32R),
                    r_sbs[hh].bitcast(F32R),
                    start=True, stop=True,
                )
                rb_sb = STR.tile([64, 512], F32R, tag="rb_sb", bufs=2)
                nc.vector.tensor_copy(rb_sb, rb_ps)
                with nc.allow_low_precision(reason="bf16 y"):
                    nc.vector.tensor_mul(
                        yt[64 * hh:64 * (hh + 1), :],
                        y_sbs[hh], rb_sb,
                    )
            lo = mq0
            while lo < mq0 + 512:
                s = lo // MS
                hi = min(mq0 + 512, (s + 1) * MS)
                nc.sync.dma_start(
                    out=a2a_in[fp][s, :, lo - s * MS:hi - s * MS],
                    in_=yt[:, lo - mq0:hi - mq0],
                )
                lo = hi

        def emit_a2a(fp):
            if os.environ.get("GQA_NO_CC"):
                nc.sync.dma_start(out=a2a_out[fp].opt(), in_=a2a_in[fp].opt())
            else:
                nc.gpsimd.collective_compute(
                    "AllToAll",
                    mybir.AluOpType.bypass,
                    replica_groups=[list(range(N_CORES))],
                    ins=[a2a_in[fp].opt()],
                    outs=[a2a_out[fp].opt()],
                )

        def emit_scores(fp, j, b, ip):
            mq0 = T * b + 512 * j
            s2 = [
                PS.tile([128, 1024], F32, tag="ps2", bufs=2, name="s2")
                for _ in range(2)
            ]
            for hh in range(2):
                for di in range(2):
                    i = 2 * ip + di
                    rel = i - 4 * j
                    # causal narrowing: for diagonal key tiles only columns
                    # q >= 128*rel survive the mask, so don't compute the
                    # rest (the stale PSUM left of q0 is finite, never read
                    # unmasked, and never fed to PV)
                    q0 = 128 * rel if 0 < rel else 0
                    mk0 = T * b + 128 * i
                    nc.tensor.matmul(
                        s2[hh][:, 512 * di + q0:512 * (di + 1)],
                        ktdup[64 * hh:64 * (hh + 1), mk0:mk0 + 128]
                        .bitcast(F32R),
                        qrope[fp][64 * hh:64 * (hh + 1),
                                  mq0 + q0:mq0 + 512].bitcast(F32R),
                        start=True, stop=True,
                        tile_position=(64 * hh, 0),
                    )
            return s2

        # flat beat stream over (fp, window, key-pair) with the scores
        # pipelined one beat ahead GLOBALLY (across window and head-pair
        # boundaries), so the scalar engine's exp stream never drains
        beats = []
        # windows ordered large-j interleaved with small-j so the per-window
        # normalization overhead (a PE matmul pair per window start) stays
        # spread out instead of bunching up in the small-window tail
        win_order = [(3, 0), (2, 0), (3, 1), (1, 0), (2, 1), (0, 0),
                     (1, 1), (0, 1)]
        assert JT == 4
        for fp in range(2):
            for j, b in win_order:
                ntk = 4 * (j + 1)
                for ip in range(ntk // 2):
                    beats.append((fp, j, b, ip, ntk))
        fp0_norms = 0
        pvps = None
        s2 = emit_scores(*beats[0][:4])
        for idx, (fp, j, b, ip, ntk) in enumerate(beats):
            mq0 = T * b + 512 * j
            s2_next = (
                emit_scores(*beats[idx + 1][:4])
                if idx + 1 < len(beats) else None
            )
            if ip == 0:
                pvps = [
                    PS.tile([HD + 1, 512], F32, tag="ps1", name="pvps")
                    for _ in range(2)
                ]
                if len(norm_pend) > 1:
                    # normalization deferred TWO windows: by the time the PE
                    # reaches the rb broadcast matmul, the DVE reciprocal
                    # chain finished long ago, so the PE stream never stalls
                    nb = norm_pend.pop(0)
                    norm_block(nb)
                    if nb[2] == 0:
                        fp0_norms += 1
                        if fp0_norms == 2 * JT:
                            emit_a2a(0)
            elif idx == len(beats) - 1 and norm_pend:
                # drain one pending normalization inside the final beat so
                # the end-of-phase flush doesn't delay the second AllToAll
                nb = norm_pend.pop(0)
                norm_block(nb)
                if nb[2] == 0:
                    fp0_norms += 1
                    if fp0_norms == 2 * JT:
                        emit_a2a(0)
            for hh in range(2):
                e_sb = EXP.tile([128, 1024], BF16, tag="e_sb")
                with nc.allow_low_precision(reason="bf16 probs"):
                    nc.scalar.activation(e_sb, s2[hh], Exp, scale=SCALE)
                for di in range(2):
                    i = 2 * ip + di
                    rel = i - 4 * j
                    if rel >= 0:
                        # rel=3 keeps the full width: it carries the
                        # accumulation-group stop, so its PV reads all 512
                        # columns and the mask must zero them all
                        q0 = 128 * rel if 0 < rel < 3 else 0
                        with nc.allow_low_precision(reason="bf16 probs"):
                            nc.vector.tensor_mul(
                                e_sb[:, 512 * di + q0:512 * (di + 1)],
                                e_sb[:, 512 * di + q0:512 * (di + 1)],
                                mask_sb[:, rel, q0:512],
                            )
                for di in range(2):
                    i = 2 * ip + di
                    rel = i - 4 * j
                    q0 = 128 * rel if 0 < rel < 3 else 0
                    gi = (T * b) // 128 + i
                    nc.tensor.matmul(
                        pvps[hh][:, q0:512], vaug[gi].bitcast(BF16),
                        e_sb[:, 512 * di + q0:512 * (di + 1)].bitcast(BF16),
                        start=(i == 0), stop=(i == ntk - 1),
                    )
            if ip == ntk // 2 - 1:
                # window done (all DVE, off the PE path): reciprocal of the
                # partition-0 denominator row, and evacuation of y to SBUF
                # so the pvps PSUM slots free without waiting for the
                # (deferred) normalization.
                r_sbs = []
                y_sbs = []
                for hh in range(2):
                    d_sb = STR.tile([1, 512], F32, tag="d_sb", bufs=2)
                    nc.vector.tensor_copy(d_sb, pvps[hh][HD:HD + 1, :])
                    r32 = STR.tile([1, 512], F32, tag="r32", bufs=2)
                    nc.vector.reciprocal_approx_fast(r32, d_sb)
                    r_sb = STR.tile([1, 512], F32R, tag="r_sb", bufs=6)
                    nc.vector.tensor_copy(r_sb, r32)
                    r_sbs.append(r_sb)
                    y_sb = STR.tile([64, 512], BF16, tag="y_sb", bufs=6)
                    with nc.allow_low_precision(reason="bf16 y"):
                        nc.vector.tensor_copy(y_sb, pvps[hh][0:HD, :])
                    y_sbs.append(y_sb)
                norm_pend.append((y_sbs, mq0, fp, r_sbs))
            s2 = s2_next
        while norm_pend:
            nb = norm_pend.pop(0)
            norm_block(nb)
            if nb[2] == 0:
                fp0_norms += 1
                if fp0_norms == 2 * JT:
                    emit_a2a(0)
        emit_a2a(1)

        # ---- phase 3: token-sharded output projection (emits out^T).
        # ytf loads on the sync ring; wo is already resident (prefetched).
        ytf = {}
        for s in range(N_CORES):
            yt_sb = P3.tile([128, MS], BF16, tag=f"ytf{2 * s}",
                            name=f"ytf{2 * s}")
            nc.sync.dma_start(out=yt_sb, in_=a2a_out[0][s, :, :])
            ytf[2 * s] = yt_sb
        # even k-tiles (from the first AllToAll): accumulate into SBUF
        # partials while the second AllToAll is in flight
        oe_sbs = []
        for n in range(NT):
            oe_ps = PS.tile([128, MS], F32, tag="ps1", name="oe_ps")
            for s in range(N_CORES):
                nc.tensor.matmul(
                    oe_ps, wo_sb[:, n, 0, s, :].bitcast(BF16),
                    ytf[2 * s].bitcast(BF16),
                    start=(s == 0), stop=(s == N_CORES - 1),
                )
            oe_sb = P3.tile([128, MS], BF16, tag=f"oe{n}", bufs=1,
                            name=f"oe{n}")
            with nc.allow_low_precision(reason="bf16 partial"):
                nc.vector.tensor_copy(oe_sb, oe_ps)
            oe_sbs.append(oe_sb)
        for s in range(N_CORES):
            yt_sb = P3.tile([128, MS], BF16, tag=f"ytf{2 * s + 1}",
                            name=f"ytf{2 * s + 1}")
            nc.sync.dma_start(out=yt_sb, in_=a2a_out[1][s, :, :])
            ytf[2 * s + 1] = yt_sb
        for n in range(NT):
            oo_ps = PS.tile([128, MS], F32, tag="ps1", name="oo_ps")
            for s in range(N_CORES):
                nc.tensor.matmul(
                    oo_ps, wo_sb[:, n, 1, s, :].bitcast(BF16),
                    ytf[2 * s + 1].bitcast(BF16),
                    start=(s == 0), stop=(s == N_CORES - 1),
                )
            ot_sb = P3.tile([128, MS], F32, tag="ot_sb", bufs=3)
            nc.vector.tensor_add(ot_sb, oo_ps, oe_sbs[n])
            nc.sync.dma_start(
                out=out.ap()[128 * n:128 * (n + 1), :], in_=ot_sb
            )
        p3ctx.close()

    nc.finalize()
    return nc


def make_inputs(x, cos, sin, wq, wk, wv, wo):
    """Host-side sharding/layout prep. Returns in_maps for the 8 cores."""
    Bx, T, _ = x.shape
    M = Bx * T
    NT = DM // 128
    xT = np.ascontiguousarray(x.reshape(M, DM).T)
    # wo packed for a single contiguous DMA into the phase-3 SBUF layout:
    # woP[p, n, two, s, c] = wo.T[s*256 + two*128 + p, n*128 + c], bf16
    woT = wo.T.reshape(8, 2, 128, NT, 128)
    woP = np.ascontiguousarray(
        woT.transpose(2, 3, 1, 0, 4).reshape(128, -1)
    ).astype(ml_dtypes.bfloat16)
    sgn = np.concatenate([-np.ones(32, np.float32), np.ones(32, np.float32)])
    cosF = np.ascontiguousarray(np.tile(cos.T, (2, 1))).astype(np.float32)
    sinF = np.ascontiguousarray(np.tile(sin.T * sgn[:, None], (2, 1))).astype(
        np.float32
    )
    pshuf = np.zeros((128, 128), np.float32)
    for m in range(128):
        pshuf[64 * (m // 64) + (m % 64 + 32) % 64, m] = 1.0
    pdup = np.zeros((64, 128), np.float32)
    pdups = np.zeros((64, 128), np.float32)
    for m in range(128):
        pdup[m % 64, m] = 1.0
        pdups[(m % 64 + 32) % 64, m] = 1.0
    p = np.arange(128)[:, None]
    q = np.arange(512)[None, :]
    cmaskM = np.stack(
        [(128 * r + p <= q).astype(np.float32) for r in range(4)]
    ).astype(ml_dtypes.bfloat16)
    in_maps = []
    for c in range(N_CORES):
        wqT = np.ascontiguousarray(wq[QF * c:QF * (c + 1), :].T)
        wkvT = np.ascontiguousarray(
            np.concatenate(
                [wk[HD * c:HD * (c + 1), :], wv[HD * c:HD * (c + 1), :]],
                axis=0,
            ).T
        )
        in_maps.append(
            {
                "xT": xT, "wqT": wqT, "wkvT": wkvT, "woP": woP,
                "cosF": cosF, "sinF": sinF, "pshuf": pshuf,
                "pdup": pdup, "pdups": pdups, "cmaskM": cmaskM,
                "identm": np.eye(64, dtype=np.float32),
                "onesm": np.ones((1, 64), np.float32),
                "vones": np.ones((128, M // 128, 1), ml_dtypes.bfloat16),
            }
        )
    return in_maps


_NC_CACHE = {}


def get_nc(T=T_FULL):
    if T not in _NC_CACHE:
        _NC_CACHE[T] = build_gqa(T)
    return _NC_CACHE[T]


def kernel(x, cos, sin, wq, wk, wv, wo, _trace=False):
    x = np.asarray(x, np.float32)
    nc = get_nc(x.shape[1])
    in_maps = make_inputs(
        x,
        np.asarray(cos, np.float32),
        np.asarray(sin, np.float32),
        np.asarray(wq, np.float32),
        np.asarray(wk, np.float32),
        np.asarray(wv, np.float32),
        np.asarray(wo, np.float32),
    )
    res = run_bass_kernel_spmd(nc, in_maps, list(range(N_CORES)), trace=_trace)
    # each core returns out^T (2048, M/8) for its disjoint token slice
    outs = [np.asarray(res.results[c]["out"]).T for c in range(N_CORES)]
    full = np.concatenate(outs, axis=0)
    Bx, T, _ = x.shape
    out = np.ascontiguousarray(full).reshape(Bx, T, DM).astype(np.float32)
    if _trace:
        return out, res
    return out
